# revision 13
# baseline (speedup 1.0000x reference)
"""Trainium2 Bass kernel for the alignment+uniformity loss.

Strategy
--------
out = mean_i ||z_i - z'_i||  +  0.5*(U(z) + U(z'))
  U(x) = log( sum_{i<j} exp(-||x_i - x_j||) / n_pairs )

The N^2 pairwise part is sharded row-wise over 8 cores.  Each core c gets
its own q-block (rows [c*B, (c+1)*B)) of z and z', plus a *rotated* copy of
the full matrices (np.roll by -c*B) so that the diagonal and the circulant
triangle schedule land at compile-time-constant positions in every core's
(identical) program.  Each unordered pair {i,j} is computed exactly once:
row r takes columns r+1..r+N/2-1 (mod N) plus weight-1/2 on column r+N/2.
Each core emits partial sums; the host combines them (a few scalars) and
applies the final log/mean.

Device pipeline per Gram tile (all matmuls fp16, fp32 PSUM accum):
  PSUM = zq . zk  - sq_k/2 (K=1 matmul)  - 30000*staircase (mask matmuls
  for the ragged triangle edges; -30000 on g => d2 += 60000 => exp == 0).
  DVE drains PSUM to fp32 strips (so the PSUM ring never waits on ACT),
  then ACT: d = Sqrt(-2*t + sq_q) -> fp16 strip;  Exp(-d) with row-sum
  accumulation.  Sqrt/Exp live in different ACT table sets, so the exp
  passes are batched 4 q-tiles per set switch.

Transposes (z -> z^T operands) are DMA work, not PE: gpsimd casting DMAs
make fp16 copies (HBM fp32 -> SBUF fp16 -> HBM fp16) and hardware xbar
transpose DMAs (on the sync HWDGE queue, which carries nothing else) land
transposed fp16 operands in SBUF, as a ring of per-chunk tiles.
"""

import sys

sys.path.insert(0, "/opt/trn_rl_repo")

import numpy as np
from contextlib import ExitStack

N, D, P = 8192, 512, 128
NCORES = 8
B = N // NCORES            # 1024 q-rows per core
QT = B // P                # 8 q-tiles per core
CH = 512                   # k-rows per chunk
KT = D // P                # 4 contraction tiles
HALF = N // 2              # 4096

TRIANGLE = True
NCHUNK = 10 if TRIANGLE else 16          # chunks of zk each core needs
NK = NCHUNK * CH                         # zk rows shipped per core
MASK_G = -30000.0                        # fp16-safe; d2 += 60000
NCHS = 9 if TRIANGLE else 16             # chunks per q-tile
ABATCH = 4                               # q-tiles per ACT table-set switch

_module = None


def _patch_act_tables():
    """Blank the `exp_and_others` / `natural_log` sets in the table list
    the ACT load-insertion pass sees, so ln/exp both resolve to the
    combined `natural_log_exp_and_others` set (ids keep their positions,
    so act_func_set_id indexing stays valid).  Without this the greedy
    pass alternates table loads (~1.5us each) between ln and exp."""
    import concourse.bacc as bacc_mod

    orig = bacc_mod.get_activation_tables
    if getattr(bacc_mod, "_aul_tables_patched", False):
        return

    def patched(arch):
        tabs = dict(orig(arch))
        for k in ("exp_and_others", "natural_log"):
            if k in tabs:
                tabs[k] = set()
        return tabs

    bacc_mod.get_activation_tables = patched
    bacc_mod._aul_tables_patched = True


def _emit(ctx, tc, nc, zq_d, zpq_d, zk_d, zpk_d, out_d):
    import concourse.bass as bass
    import concourse.tile as tile
    from concourse import mybir

    f32 = mybir.dt.float32
    f16 = mybir.dt.float16
    AF = mybir.ActivationFunctionType
    OP = mybir.AluOpType

    consts = ctx.enter_context(tc.tile_pool(name="consts", bufs=1))
    resident = ctx.enter_context(tc.tile_pool(name="resident", bufs=1))
    scratch = ctx.enter_context(tc.tile_pool(name="scratch", bufs=2))
    stg_pool = ctx.enter_context(tc.tile_pool(name="stg_pool", bufs=3))
    ring_pool = ctx.enter_context(tc.tile_pool(name="ring_pool", bufs=1))
    psum = ctx.enter_context(tc.tile_pool(name="psum", bufs=2, space="PSUM"))

    ones64 = consts.tile([1, P], f16)
    nc.vector.memset(ones64, 64.0)

    # --- triangle staircase masks as matmul constants -------------------
    # mask contribution M[p, f] = lhsT_tri.T @ B[v]; applied inside the
    # PSUM accumulation group of the ragged start/end chunks.
    # lo (start chunk): kill f <= v*128 + p ; hi (end chunk): kill f >= off.
    a_lo = consts.tile([P, P], f16)     # A_lo[k, p] = 1 where k <= p
    nc.gpsimd.memset(a_lo, 0.0)
    nc.gpsimd.affine_select(
        out=a_lo, in_=a_lo, compare_op=OP.is_gt, fill=1.0,
        base=0, pattern=[[-1, P]], channel_multiplier=1,
    )  # keep 0 where k - f... iota = k*1 + f*(-1) > 0 -> keep(0) where k>f, fill 1 where k<=f
    a_hi = consts.tile([P, P], f16)     # A_hi[k, p] = 1 where k >= p
    nc.gpsimd.memset(a_hi, 0.0)
    nc.gpsimd.affine_select(
        out=a_hi, in_=a_hi, compare_op=OP.is_gt, fill=1.0,
        base=0, pattern=[[1, P]], channel_multiplier=-1,
    )  # iota = f - k > 0 -> keep(0) where f>k, fill 1 where k>=f... (k partition, f free)
    b_lo, b_hi = [], []
    for v in range(4):
        bl = consts.tile([P, CH], f16, name=f"b_lo{v}")
        nc.gpsimd.memset(bl, 0.0)
        # shifted identity: -M where f == k + v*128
        nc.gpsimd.affine_select(
            out=bl, in_=bl, compare_op=OP.not_equal, fill=MASK_G,
            base=-(v * P), pattern=[[1, CH]], channel_multiplier=-1,
        )
        if v > 0:   # cols below the staircase block: killed for every p (k=0 row)
            nc.gpsimd.affine_select(
                out=bl[:, 0 : v * P], in_=bl[:, 0 : v * P],
                compare_op=OP.not_equal, fill=MASK_G,
                base=0, pattern=[[0, v * P]], channel_multiplier=1,
            )
        b_lo.append(bl)
        bh = consts.tile([P, CH], f16, name=f"b_hi{v}")
        nc.gpsimd.memset(bh, 0.0)
        nc.gpsimd.affine_select(
            out=bh, in_=bh, compare_op=OP.not_equal, fill=MASK_G,
            base=-(v * P), pattern=[[1, CH]], channel_multiplier=-1,
        )
        if v * P + P < CH:  # cols above the staircase block (k=127 row)
            nc.gpsimd.affine_select(
                out=bh[:, v * P + P : CH], in_=bh[:, v * P + P : CH],
                compare_op=OP.not_equal, fill=MASK_G,
                base=-(P - 1), pattern=[[0, CH - v * P - P]],
                channel_multiplier=1,
            )
        b_hi.append(bh)

    # --- phase A: q blocks (fp32), norms, alignment, gap-N/2 pairs ------
    sqq = []       # fp32 [P, QT] per matrix (exact row norms of q rows)
    zqT = []       # fp16 [P, KT, B] per matrix (via casting DMA + xbar)
    align_acc = resident.tile([P, 1], f32)
    gap_acc = []   # [P,1] per matrix: 0.5 * sum exp(-d) over gap-N/2 pairs
    with tc.tile_pool(name="phasea", bufs=1) as phasea, tc.tile_pool(
        name="phasea_sc", bufs=2
    ) as phasea_sc:
        zq_sb = []
        for m, src in enumerate((zq_d, zpq_d)):
            zsb = phasea.tile([P, QT, D], f32, name=f"zq_sb{m}")
            nc.scalar.dma_start(
                out=zsb, in_=src.rearrange("(t p) d -> p t d", p=P)
            )
            zq_sb.append(zsb)

            sq = resident.tile([P, QT], f32, name=f"sqq{m}")
            for t in range(QT):
                dum = phasea_sc.tile([P, D], f32, tag="dum", name="dum")
                nc.vector.scalar_tensor_tensor(
                    out=dum, in0=zsb[:, t], scalar=1.0, in1=zsb[:, t],
                    op0=OP.mult, op1=OP.mult, accum_out=sq[:, t : t + 1],
                )
            sqq.append(sq)

            # fp16 copy of the q block (casting DMA) -> xbar transpose
            zq16_dram = nc.dram_tensor(f"zq16_dram{m}", [B, D], f16).ap()
            nc.gpsimd.dma_start(out=zq16_dram, in_=src)
            zt = resident.tile([P, KT, B], f16, name=f"zqT{m}")
            for kt in range(KT):
                nc.sync.dma_start(
                    out=zt[:, kt],
                    in_=zq16_dram[:, kt * P : (kt + 1) * P],
                    transpose=True,
                )
            zqT.append(zt)

        # alignment: d = sqrt(||zq - zpq||^2) row-wise
        a2 = phasea_sc.tile([P, QT], f32, tag="small8")
        for t in range(QT):
            diff = phasea_sc.tile([P, D], f32, tag="dum", name="diff")
            nc.vector.tensor_sub(diff, zq_sb[0][:, t], zq_sb[1][:, t])
            dum2 = phasea_sc.tile([P, D], f32, tag="dum", name="dum2")
            nc.vector.scalar_tensor_tensor(
                out=dum2, in0=diff, scalar=1.0, in1=diff,
                op0=OP.mult, op1=OP.mult, accum_out=a2[:, t : t + 1],
            )
        da = phasea_sc.tile([P, QT], f32, tag="small8", name="da")
        nc.scalar.activation(da, a2, AF.Sqrt)
        nc.vector.tensor_reduce(
            align_acc, da, axis=mybir.AxisListType.X, op=OP.add
        )

        # gap-N/2 pairs (local row r vs rotated row r+4096), weight 1/2
        if TRIANGLE:
            for m, ksrc in enumerate((zk_d, zpk_d)):
                g2 = phasea_sc.tile([P, QT], f32, tag="small8", name=f"g2_{m}")
                for half in range(2):
                    gstg = phasea_sc.tile(
                        [P, 4, CH], f32, tag="gstg", name="gstg"
                    )
                    nc.scalar.dma_start(
                        out=gstg,
                        in_=ksrc[
                            (8 + half) * CH : (9 + half) * CH, :
                        ].rearrange("(r p) d -> p r d", p=P),
                    )
                    for r in range(4):
                        t = half * 4 + r
                        gdiff = phasea_sc.tile(
                            [P, D], f32, tag="dum", name="gdiff"
                        )
                        nc.vector.tensor_sub(
                            gdiff, zq_sb[m][:, t], gstg[:, r]
                        )
                        gdum = phasea_sc.tile(
                            [P, D], f32, tag="dum", name="gdum"
                        )
                        nc.vector.scalar_tensor_tensor(
                            out=gdum, in0=gdiff, scalar=1.0, in1=gdiff,
                            op0=OP.mult, op1=OP.mult,
                            accum_out=g2[:, t : t + 1],
                        )
                dg = phasea_sc.tile([P, QT], f32, tag="small8", name="dg")
                nc.scalar.activation(dg, g2, AF.Sqrt)
                vg = phasea_sc.tile([P, QT], f32, tag="small8", name="vg")
                gac = resident.tile([P, 1], f32, name=f"gap_acc{m}")
                nc.scalar.activation(vg, dg, AF.Exp, scale=-1.0, accum_out=gac)
                gap_acc.append(gac)

    # --- B1 per matrix: casting loads, row norms, fp16 bounce, xbar -----
    # zkT ring: per-chunk transposed tiles [P, KT, CH], shared slot ring
    # across both matrices.
    zkT = {}       # (m, ch) -> tile
    rhs2 = []      # [1, NK] fp16 per matrix: -sq_k/128
    for m, ksrc in enumerate((zk_d, zpk_d)):
        zk16_dram = nc.dram_tensor(f"zk16_dram{m}", [NK, D], f16).ap()
        sqk = scratch.tile([P, NCHUNK * 4], f32, tag="sqk", name=f"sqk{m}")
        for ch in range(NCHUNK):
            stg16 = stg_pool.tile([P, 4, CH], f16, tag="stg", name="stg16")
            nc.gpsimd.dma_start(
                out=stg16,
                in_=ksrc[ch * CH : (ch + 1) * CH, :].rearrange(
                    "(r p) d -> p r d", p=P
                ),
            )
            for r in range(4):
                dumk = scratch.tile([P, CH], f16, tag="dumk", name="dumk")
                nc.vector.scalar_tensor_tensor(
                    out=dumk, in0=stg16[:, r], scalar=1.0, in1=stg16[:, r],
                    op0=OP.mult, op1=OP.mult,
                    accum_out=sqk[:, ch * 4 + r : ch * 4 + r + 1],
                )
            nc.gpsimd.dma_start(
                out=zk16_dram[ch * CH : (ch + 1) * CH, :].rearrange(
                    "(r p) d -> p r d", p=P
                ),
                in_=stg16,
            )
            zt = ring_pool.tile(
                [P, KT, CH], f16, tag="zkTc", bufs=NCHS + 2,
                name=f"zkT{m}_{ch}",
            )
            for kt in range(KT):
                nc.sync.dma_start(
                    out=zt[:, kt],
                    in_=zk16_dram[
                        ch * CH : (ch + 1) * CH, kt * P : (kt + 1) * P
                    ],
                    transpose=True,
                )
            zkT[(m, ch)] = zt

        rhs2.append((sqk, None))

    # rhs2[0, j] = -sq_k[j] / 128  (fp16), bounced through DRAM per
    # 4-chunk group so the first matmul group doesn't wait on all of B1.
    # The tiny DMAs ride the otherwise-idle ACT HWDGE queue.
    for m in range(2):
        sqk = rhs2[m][0]
        sqk16 = scratch.tile(
            [P, NCHUNK * 4], f16, tag="sqk16", name=f"sqk16{m}"
        )
        sq_dram = nc.dram_tensor(f"sq_bounce{m}", [NK], f16).ap()
        r2 = resident.tile([1, NK], f16, name=f"rhs2_{m}")
        for c0, c1 in ((0, 4), (4, 8), (8, NCHUNK)):
            nc.vector.tensor_scalar_mul(
                sqk16[:, c0 * 4 : c1 * 4], sqk[:, c0 * 4 : c1 * 4],
                -1.0 / 128.0,
            )
            nc.scalar.dma_start(
                out=sq_dram[c0 * CH : c1 * CH].rearrange("(c p) -> p c", p=P),
                in_=sqk16[:, c0 * 4 : c1 * 4],
            )
            nc.scalar.dma_start(
                out=r2[:, c0 * CH : c1 * CH],
                in_=sq_dram[c0 * CH : c1 * CH].rearrange(
                    "(o n) -> o n", o=1
                ),
            )
        rhs2[m] = r2

    # --- main pass: 16 q-tiles (2 matrices x 8), ACT batched by ABATCH --
    strip_t = ctx.enter_context(tc.tile_pool(name="strip_t", bufs=2))
    strip_d = ctx.enter_context(tc.tile_pool(name="strip_d", bufs=ABATCH))
    acc_m = []
    for m in range(2):
        acc = resident.tile([P, 1], f32, name=f"acc{m}")
        nc.vector.memset(acc, 0.0)
        acc_m.append(acc)

    tiles = [(m, t) for m in range(2) for t in range(QT)]
    for b0 in range(0, len(tiles), ABATCH):
        batch = tiles[b0 : b0 + ABATCH]
        d_strips = []
        for m, t in batch:
            chs = (
                list(range(t // 4, t // 4 + NCHS))
                if TRIANGLE
                else list(range(16))
            )
            ncols = len(chs) * CH
            t_strip = strip_t.tile(
                [P, ncols], f32, tag="t_strip", name="t_strip"
            )
            d_strip = strip_d.tile(
                [P, ncols], f16, tag="d_strip", name="d_strip"
            )
            col = 0
            for g0 in range(0, len(chs), 4):
                grp = chs[g0 : g0 + 4]
                gw = len(grp) * CH
                gp = psum.tile([P, 2048], f32, tag="ps", name="gp")
                for gi, ch in enumerate(grp):
                    sl = gp[:, gi * CH : (gi + 1) * CH]
                    for kt in range(KT):
                        nc.tensor.matmul(
                            sl,
                            lhsT=zqT[m][:, kt, t * P : (t + 1) * P],
                            rhs=zkT[(m, ch)][:, kt],
                            start=(kt == 0),
                            stop=False,
                        )
                    last = True
                    if TRIANGLE and ch == t // 4:
                        last = False
                    if TRIANGLE and ch == t // 4 + NCHS - 1:
                        last = False
                    nc.tensor.matmul(
                        sl,
                        lhsT=ones64,
                        rhs=rhs2[m][:, ch * CH : (ch + 1) * CH],
                        start=False,
                        stop=last,
                    )
                    if TRIANGLE and ch == t // 4:      # ragged start chunk
                        nc.tensor.matmul(
                            sl, lhsT=a_lo, rhs=b_lo[t % 4],
                            start=False, stop=True,
                        )
                    if TRIANGLE and ch == t // 4 + NCHS - 1:  # ragged end
                        nc.tensor.matmul(
                            sl, lhsT=a_hi, rhs=b_hi[t % 4],
                            start=False, stop=True,
                        )
                # DVE drains PSUM so the PSUM ring never waits on ACT
                nc.vector.tensor_copy(t_strip[:, col : col + gw], gp[:, :gw])
                col += gw
            # d = sqrt(-2*t + sq_q)   (fp16 out)
            nc.scalar.activation(
                d_strip, t_strip, AF.Sqrt,
                bias=sqq[m][:, t : t + 1], scale=-2.0,
            )
            d_strips.append((m, d_strip))
        for m, d_strip in d_strips:
            dummy = scratch.tile(
                [P, d_strip.shape[1]], f16, tag="edum", name="edum"
            )
            acc_t = scratch.tile([P, 1], f32, tag="acc_t", name="acc_t")
            nc.scalar.activation(
                dummy, d_strip, AF.Exp, scale=-1.0, accum_out=acc_t
            )
            nc.vector.tensor_add(acc_m[m], acc_m[m], acc_t)

    for m in range(2):
        if TRIANGLE:
            # acc += 0.5 * gap_acc
            nc.vector.scalar_tensor_tensor(
                out=acc_m[m], in0=gap_acc[m], scalar=0.5, in1=acc_m[m],
                op0=OP.mult, op1=OP.add,
            )

    out_sb = consts.tile([P, 4], f32)
    nc.vector.memset(out_sb, 0.0)
    nc.vector.tensor_copy(out_sb[:, 0:1], acc_m[0])
    nc.vector.tensor_copy(out_sb[:, 1:2], acc_m[1])
    nc.vector.tensor_copy(out_sb[:, 2:3], align_acc)
    nc.gpsimd.dma_start(out=out_d, in_=out_sb)


def _build():
    import concourse.bacc as bacc
    import concourse.tile as tile
    from concourse import mybir

    _patch_act_tables()
    f32 = mybir.dt.float32
    nc = bacc.Bacc(
        "TRN2", debug=False, target_bir_lowering=False, num_devices=NCORES
    )
    zq_d = nc.dram_tensor("zq", [B, D], f32, kind="ExternalInput").ap()
    zpq_d = nc.dram_tensor("zpq", [B, D], f32, kind="ExternalInput").ap()
    zk_d = nc.dram_tensor("zk", [NK, D], f32, kind="ExternalInput").ap()
    zpk_d = nc.dram_tensor("zpk", [NK, D], f32, kind="ExternalInput").ap()
    out_d = nc.dram_tensor("acc", [P, 4], f32, kind="ExternalOutput").ap()

    with tile.TileContext(nc) as tc, ExitStack() as ctx:
        _emit(ctx, tc, nc, zq_d, zpq_d, zk_d, zpk_d, out_d)
    nc.compile()
    return nc


def _get_module():
    global _module
    if _module is None:
        _module = _build()
    return _module


def _in_maps(z, zp):
    maps = []
    for c in range(NCORES):
        zrot = np.roll(z, -c * B, axis=0)[:NK]
        zprot = np.roll(zp, -c * B, axis=0)[:NK]
        maps.append(
            {
                "zq": np.ascontiguousarray(z[c * B : (c + 1) * B]),
                "zpq": np.ascontiguousarray(zp[c * B : (c + 1) * B]),
                "zk": np.ascontiguousarray(zrot),
                "zpk": np.ascontiguousarray(zprot),
            }
        )
    return maps


def _combine(accs):
    S_z = sum(float(a[:, 0].sum(dtype=np.float64)) for a in accs)
    S_zp = sum(float(a[:, 1].sum(dtype=np.float64)) for a in accs)
    align = sum(float(a[:, 2].sum(dtype=np.float64)) for a in accs) / N
    n_pairs = N * (N - 1) / 2.0
    if TRIANGLE:
        unif = 0.5 * (np.log(S_z / n_pairs) + np.log(S_zp / n_pairs))
    else:
        unif = 0.5 * (np.log(S_z / (2 * n_pairs)) + np.log(S_zp / (2 * n_pairs)))
    return np.float32(align + unif)


def kernel(z, z_prime, _trace=False, _tmpdir=None):
    from concourse.bass_utils import run_bass_kernel_spmd

    z = np.ascontiguousarray(np.asarray(z, dtype=np.float32))
    zp = np.ascontiguousarray(np.asarray(z_prime, dtype=np.float32))
    assert z.shape == (N, D) and zp.shape == (N, D)
    nc = _get_module()
    res = run_bass_kernel_spmd(
        nc, _in_maps(z, zp), list(range(NCORES)), trace=_trace, tmpdir=_tmpdir
    )
    out = _combine([res.results[c]["acc"] for c in range(NCORES)])
    if _trace:
        return out, res
    return out
